# revision 33
# baseline (speedup 1.0000x reference)
"""Binarized 3x3 conv (stride 1, pad 1) + bias on 8 Trainium2 NeuronCores.

Full problem: x[32,256,56,56] f32, weight[256,256,3,3] f32, bias[256] f32
-> y[32,256,56,56] f32 with y = conv2d(sign(x), sign(weight), pad=1) + bias
(sign(t) = +1 for t >= 0 else -1).

Sharding: data-parallel over batch. Each of the 8 cores gets 4 images and a
replicated copy of weight/bias, computes its shard fully on-device, and the
host concatenates the 8 output shards.

Per-core kernel:
  - binarize x and w to +/-0.5 with one fused DVE op each ((v>=0) - 0.5);
    the final PSUM->SBUF copy applies scale=4 to undo the 0.25 product
    scale, so results are exactly the +/-1 conv (all integers, exact in f32).
  - x lives zero-padded in SBUF as [128(ci_p), 2(ci_blk), 3312] fp8 per
    image in a shared-pad 57-wide row layout: row q (q=0..57, image row q-1)
    at elems 57q..57q+56; col 0 is the left pad and doubles as the previous
    row's right pad. Pad zeros are written once per buffer; data rows are
    rewritten per image.
  - weight is binarized to bf16, transposed on the PE (36 x 128x128
    transposes via identity), and stored as fp8 lhsT
    [128(ci_p), 2(ci_blk), 9(tap), 256(co)].
  - conv: for each (co_blk, image, 8-row output chunk): accumulate 9
    DoubleRow fp8 matmuls (one per tap, K=256 packed as [128,2]),
    M=128/N=455, into one PSUM bank; issue rate is N/2.4GHz + ~2.5ns with
    LDWEIGHTS fully hidden. One garbage column per row is skipped on the
    way out.
  - PSUM -> SBUF via ScalarE: Identity(psum*4 + bias[co]), then DMA to y.
  - the chip power governor only grants sustained full PE clock ~29us in;
    fp8-DR junk matmuls on zeros from t~0 (never bf16 -- that triggers a
    half-clock payback window) plus a k-major chunk order for image 0 keep
    the PE busy through the ramp so most of it is spent on real work.
"""

import numpy as np

import concourse.bacc as bacc
import concourse.mybir as mybir
import concourse.tile as tile
from concourse.bass_utils import run_bass_kernel_spmd
from concourse.masks import make_identity

F32 = mybir.dt.float32
BF16 = mybir.dt.bfloat16
FP8 = mybir.dt.float8e4
AF = mybir.ActivationFunctionType
ALU = mybir.AluOpType
DR = mybir.MatmulPerfMode.DoubleRow

N_CORES = 8
H = W = 56
WP = 58            # padded row width
CIN = 256
COUT = 256
CI_BLKS = 2        # 256 ci = 2 x 128 partitions
CO_BLKS = 2
R = 8              # output rows per chunk
NCHUNK = H // R    # 7
NV = R * WP        # 464 matmul moving free size
IMG_FA = 3376      # aligned per-ci_blk padded image elems (58*58+2 -> 3376)
WP2 = 57           # shared-pad row width: [pad][56 data], right pad = next row's col 0
NV2 = 455          # 7*57+55+1 moving positions per 8-row chunk
FA2 = 3312         # 57*58 rows + 1 guard, 16-aligned


def _build_conv(tc, y_ap, x_ap, w_ap, b_ap, n_imgs):
    nc = tc.nc
    scale = 4.0  # undo (+/-0.5)*(+/-0.5) = +/-0.25 product scale

    with (
        tc.tile_pool(name="consts", bufs=1) as consts,
        tc.tile_pool(name="wstage", bufs=1) as wstage_pool,
        tc.tile_pool(name="lhst", bufs=1) as lhst_pool,
        tc.tile_pool(name="xstage", bufs=2) as xstage_pool,
        tc.tile_pool(name="xpad", bufs=1) as xpad_pool,
        tc.tile_pool(name="outsb", bufs=3) as out_pool,
        tc.tile_pool(name="tpsum", bufs=2, space="PSUM") as tpsum_pool,
        tc.tile_pool(name="cpsum", bufs=4, space="PSUM") as cpsum_pool,
    ):
        # --- constants -----------------------------------------------------
        # fp8-DR junk matmuls on zeros from t~0: the HAM power governor grants
        # full PE clock only after ~10us of sustained utilization, so start
        # the clock before any DMA lands (bf16 junk would trigger a payback
        # half-clock window mid-conv; the fp8-DR stream does not).
        junk = consts.tile([128, 2, 512], FP8, name="junk")
        nc.vector.memset(junk, 0.0)
        junk_lhs = consts.tile([128, 2, 128], FP8, name="junk_lhs")
        nc.vector.memset(junk_lhs, 0.0)
        ident = consts.tile([128, 128], BF16)
        make_identity(nc, ident)

        def junk_mm():
            jps = cpsum_pool.tile([128, 512], F32, name="ps", tag="ps")
            nc.tensor.matmul(jps, junk_lhs, junk, start=True, stop=True,
                             perf_mode=DR)

        for _ in range(6):
            junk_mm()

        # --- DMA issue order is bandwidth-critical: the conv stream can't
        # start until W_c0 + the first x rows are in SBUF (~360 GB/s/core).
        wstage = wstage_pool.tile([128, CO_BLKS, CIN, 9], F32)
        wb = wstage_pool.tile([128, CO_BLKS, CIN, 9], BF16)
        lhst = lhst_pool.tile([128, CI_BLKS, 9, COUT], FP8)
        xstage0 = xstage_pool.tile([128, CI_BLKS, H * W], F32,
                                   name="xstage0", tag="xstage")

        def dma_w(c, b):
            # one quarter of the weights: co block c, ci block b
            nc.sync.dma_start(
                out=wstage[:, c, b * 128:(b + 1) * 128],
                in_=w_ap[c * 128:(c + 1) * 128, b * 128:(b + 1) * 128].rearrange(
                    "co ci kh kw -> co ci (kh kw)"),
            )

        def dma_x(xstage, n, r0, r1, b):
            nc.sync.dma_start(
                out=xstage[:, b, r0 * W:r1 * W],
                in_=x_ap[n, b * 128:(b + 1) * 128, r0:r1]
                    .rearrange("c h w -> c (h w)"),
            )

        # interleave so the bytes gating the first conv chunk arrive first:
        # lhsT(c=0) needs both W_c0 quarters; chunk k=0 needs x rows 0-8 only
        dma_w(0, 0)
        dma_w(0, 1)
        dma_x(xstage0, 0, 0, 9, 0)
        dma_x(xstage0, 0, 0, 9, 1)
        dma_w(1, 0)
        dma_w(1, 1)
        dma_x(xstage0, 0, 9, 18, 0)
        dma_x(xstage0, 0, 9, 18, 1)
        dma_x(xstage0, 0, 18, 28, 0)
        dma_x(xstage0, 0, 18, 28, 1)
        dma_x(xstage0, 0, 28, 42, 0)
        dma_x(xstage0, 0, 28, 42, 1)
        dma_x(xstage0, 0, 42, H, 0)
        dma_x(xstage0, 0, 42, H, 1)
        bias_sb = consts.tile([128, CO_BLKS], F32)
        nc.scalar.dma_start(out=bias_sb, in_=b_ap.rearrange("(b p) -> p b", p=128))

        # --- weight prep (PSUM->SBUF casts on ScalarE); DVE work emitted in
        # DMA-arrival order ---------------------------------------------------
        # lhsT[ci_p, ci_blk, tap, co] in fp8 (cast on the PSUM->SBUF copy)
        def binz(dst, src):
            nc.vector.tensor_scalar(dst, src, 0.0, 0.5, ALU.is_ge, ALU.subtract)

        def wprep(c, b):
            binz(wb[:, c, b * 128:(b + 1) * 128], wstage[:, c, b * 128:(b + 1) * 128])
            for t in range(9):
                tp = tpsum_pool.tile([128, 128], BF16)
                nc.tensor.transpose(tp, wb[:, c, b * 128:(b + 1) * 128, t], ident)
                nc.scalar.copy(out=lhst[:, b, t, c * 128:(c + 1) * 128],
                               in_=tp)

        # --- x buffers: persistent padded buffers, pad zeros written once
        NXPAD = 3
        xpads = [xpad_pool.tile([128, CI_BLKS, FA2], FP8,
                                name=f"xpad{i}", tag=f"xpad{i}")
                 for i in range(NXPAD)]
        for xp in xpads:
            for b in range(CI_BLKS):
                # top pad row q=0 + left pad of row q=1
                nc.vector.memset(xp[:, b, 0:58], 0.0)
                # bottom pad row q=57 + tail guard/align
                nc.vector.memset(xp[:, b, 57 * 57:FA2], 0.0)
                # left-pad col (elem 57q) of rows q=1..56; doubles as the
                # previous row's right pad in the shared-pad layout
                nc.vector.memset(
                    xp[:, b, 57:57 + 56 * WP2].rearrange(
                        "p (h w) -> p h w", w=WP2)[:, :, 0:1],
                    0.0,
                )

        # --- per-image pipeline -------------------------------------------
        def binz_x(xstage, xpad, r0, r1, b):
            # data row h -> elems 58+57h .. 58+57h+55
            dst = xpad[:, b, 58:58 + H * WP2].rearrange(
                "p (h w) -> p h w", w=WP2)[:, r0:r1, 0:W]
            src = xstage[:, b].rearrange("p (h w) -> p h w", w=W)[:, r0:r1]
            binz(dst, src)

        def conv_chunk(n, xpad, c, k):
            ps = cpsum_pool.tile([128, 456], F32, name="ps", tag="ps")
            for t in range(9):
                kh, kw = divmod(t, 3)
                base = (R * k + kh) * WP2 + kw
                nc.tensor.matmul(
                    ps[:, 0:NV2],
                    lhst[:, 0:2, t, c * 128:(c + 1) * 128],
                    xpad[:, 0:2, base:base + NV2],
                    start=(t == 0),
                    stop=(t == 8),
                    perf_mode=DR,
                )
            osb = out_pool.tile([128, R * W], F32, name="osb")
            nc.scalar.activation(
                out=osb.rearrange("p (r w) -> p r w", w=W),
                in_=ps.rearrange("p (r w) -> p r w", w=WP2)[:, :, 0:56],
                func=AF.Identity,
                bias=bias_sb[:, c:c + 1],
                scale=scale,
            )
            nc.sync.dma_start(
                out=y_ap[n, c * 128:(c + 1) * 128]
                    .rearrange("co h w -> co (h w)")[:, R * W * k:R * W * (k + 1)],
                in_=osb,
            )

        def load_image(n):
            # loads + binarizes image n into its xpad buffer
            xstage = xstage_pool.tile([128, CI_BLKS, H * W], F32,
                                      name=f"xstage{n}", tag="xstage")
            xpad = xpads[n % NXPAD]
            for r0, r1 in ((0, 28), (28, H)):
                for b in range(CI_BLKS):
                    dma_x(xstage, n, r0, r1, b)
                    binz_x(xstage, xpad, r0, r1, b)

        for n in range(n_imgs):
            xpad = xpads[n % NXPAD]
            if n == 0:
                # emission order = engine program order (DMA-arrival order);
                # junk matmuls between wprep groups run in the PE-queue slots
                # where transposes would stall on the DVE binarize, keeping
                # early PE utilization high for the power governor
                wprep(0, 0)
                junk_mm()
                junk_mm()
                wprep(0, 1)
                junk_mm()
                junk_mm()
                binz_x(xstage0, xpad, 0, 9, 0)
                binz_x(xstage0, xpad, 0, 9, 1)
                wprep(1, 0)
                junk_mm()
                junk_mm()
                wprep(1, 1)
                junk_mm()
                junk_mm()
                binz_x(xstage0, xpad, 9, 18, 0)
                binz_x(xstage0, xpad, 9, 18, 1)
                binz_x(xstage0, xpad, 18, 28, 0)
                binz_x(xstage0, xpad, 18, 28, 1)
                binz_x(xstage0, xpad, 28, 42, 0)
                binz_x(xstage0, xpad, 28, 42, 1)
                binz_x(xstage0, xpad, 42, H, 0)
                binz_x(xstage0, xpad, 42, H, 1)
                # no HAM warm-up: sustained fp8-DR conv never triggers the
                # power governor's payback throttle (bf16 junk did), and
                # starting conv early at the cold half-clock beats burning
                # ~8us of junk matmuls first.
            # prefetch image n+1 before image n's conv chunks so its input
            # DMAs take queue priority over image n's output-DMA burst
            if n + 1 < n_imgs:
                load_image(n + 1)
            if n == 0:
                # k-major: image 0's rows are still streaming in from HBM;
                # visiting each row chunk twice (both co blocks) halves the
                # row-consumption rate so the PE never starves behind the DMA
                for k in range(NCHUNK):
                    for c in range(CO_BLKS):
                        conv_chunk(n, xpad, c, k)
            else:
                for c in range(CO_BLKS):
                    for k in range(NCHUNK):
                        conv_chunk(n, xpad, c, k)


ALU = mybir.AluOpType

_NC_CACHE = {}


def _get_nc(n_imgs):
    if n_imgs not in _NC_CACHE:
        nc = bacc.Bacc("TRN2", target_bir_lowering=False, debug=False)
        x_ap = nc.dram_tensor("x", [n_imgs, CIN, H, W], F32,
                              kind="ExternalInput").ap()
        w_ap = nc.dram_tensor("weight", [COUT, CIN, 3, 3], F32,
                              kind="ExternalInput").ap()
        b_ap = nc.dram_tensor("bias", [COUT], F32, kind="ExternalInput").ap()
        y_ap = nc.dram_tensor("y", [n_imgs, COUT, H, W], F32,
                              kind="ExternalOutput").ap()
        with tile.TileContext(nc) as tc:
            _build_conv(tc, y_ap, x_ap, w_ap, b_ap, n_imgs)
        nc.compile()
        _NC_CACHE[n_imgs] = nc
    return _NC_CACHE[n_imgs]


def kernel(x: np.ndarray, weight: np.ndarray, bias: np.ndarray) -> np.ndarray:
    assert x.shape[1:] == (CIN, H, W), x.shape
    assert x.shape[0] % N_CORES == 0, x.shape
    n_imgs = x.shape[0] // N_CORES
    x = np.ascontiguousarray(x, dtype=np.float32)
    weight = np.ascontiguousarray(weight, dtype=np.float32)
    bias = np.ascontiguousarray(bias, dtype=np.float32)

    nc = _get_nc(n_imgs)
    shards = [x[i * n_imgs:(i + 1) * n_imgs] for i in range(N_CORES)]
    in_maps = [{"x": s, "weight": weight, "bias": bias} for s in shards]
    res = run_bass_kernel_spmd(nc, in_maps, core_ids=list(range(N_CORES)))
    return np.concatenate([r["y"] for r in res.results], axis=0)



# revision 34
# speedup vs baseline: 1.0072x; 1.0072x over previous
"""Binarized 3x3 conv (stride 1, pad 1) + bias on 8 Trainium2 NeuronCores.

Full problem: x[32,256,56,56] f32, weight[256,256,3,3] f32, bias[256] f32
-> y[32,256,56,56] f32 with y = conv2d(sign(x), sign(weight), pad=1) + bias
(sign(t) = +1 for t >= 0 else -1).

Sharding: data-parallel over batch. Each of the 8 cores gets 4 images and a
replicated copy of weight/bias, computes its shard fully on-device, and the
host concatenates the 8 output shards.

Per-core kernel:
  - binarize x and w to +/-0.5 with one fused DVE op each ((v>=0) - 0.5);
    the final PSUM->SBUF copy applies scale=4 to undo the 0.25 product
    scale, so results are exactly the +/-1 conv (all integers, exact in f32).
  - x lives zero-padded in SBUF as [128(ci_p), 2(ci_blk), 3312] fp8 per
    image in a shared-pad 57-wide row layout: row q (q=0..57, image row q-1)
    at elems 57q..57q+56; col 0 is the left pad and doubles as the previous
    row's right pad. Pad zeros are written once per buffer; data rows are
    rewritten per image.
  - weight is binarized to bf16, transposed on the PE (36 x 128x128
    transposes via identity), and stored as fp8 lhsT
    [128(ci_p), 2(ci_blk), 9(tap), 256(co)].
  - conv: for each (co_blk, image, 8-row output chunk): accumulate 9
    DoubleRow fp8 matmuls (one per tap, K=256 packed as [128,2]),
    M=128/N=455, into one PSUM bank; issue rate is N/2.4GHz + ~2.5ns with
    LDWEIGHTS fully hidden. One garbage column per row is skipped on the
    way out.
  - PSUM -> SBUF via ScalarE: Identity(psum*4 + bias[co]), then DMA to y.
  - the chip power governor only grants sustained full PE clock ~29us in;
    fp8-DR junk matmuls on zeros from t~0 (never bf16 -- that triggers a
    half-clock payback window) plus a k-major chunk order for image 0 keep
    the PE busy through the ramp so most of it is spent on real work.
"""

import numpy as np

import concourse.bacc as bacc
import concourse.mybir as mybir
import concourse.tile as tile
from concourse.bass_utils import run_bass_kernel_spmd
from concourse.masks import make_identity

F32 = mybir.dt.float32
BF16 = mybir.dt.bfloat16
FP8 = mybir.dt.float8e4
AF = mybir.ActivationFunctionType
ALU = mybir.AluOpType
DR = mybir.MatmulPerfMode.DoubleRow

N_CORES = 8
H = W = 56
WP = 58            # padded row width
CIN = 256
COUT = 256
CI_BLKS = 2        # 256 ci = 2 x 128 partitions
CO_BLKS = 2
R = 8              # output rows per chunk
NCHUNK = H // R    # 7
NV = R * WP        # 464 matmul moving free size
IMG_FA = 3376      # aligned per-ci_blk padded image elems (58*58+2 -> 3376)
WP2 = 57           # shared-pad row width: [pad][56 data], right pad = next row's col 0
NV2 = 455          # 7*57+55+1 moving positions per 8-row chunk
FA2 = 3312         # 57*58 rows + 1 guard, 16-aligned


def _build_conv(tc, y_ap, x_ap, w_ap, b_ap, n_imgs):
    nc = tc.nc
    scale = 4.0  # undo (+/-0.5)*(+/-0.5) = +/-0.25 product scale

    with (
        tc.tile_pool(name="consts", bufs=1) as consts,
        tc.tile_pool(name="wstage", bufs=1) as wstage_pool,
        tc.tile_pool(name="lhst", bufs=1) as lhst_pool,
        tc.tile_pool(name="xstage", bufs=2) as xstage_pool,
        tc.tile_pool(name="xpad", bufs=1) as xpad_pool,
        tc.tile_pool(name="outsb", bufs=3) as out_pool,
        tc.tile_pool(name="tpsum", bufs=2, space="PSUM") as tpsum_pool,
        tc.tile_pool(name="cpsum", bufs=4, space="PSUM") as cpsum_pool,
    ):
        # --- constants -----------------------------------------------------
        # fp8-DR junk matmuls on zeros from t~0: the HAM power governor grants
        # full PE clock only after ~10us of sustained utilization, so start
        # the clock before any DMA lands (bf16 junk would trigger a payback
        # half-clock window mid-conv; the fp8-DR stream does not).
        junk = consts.tile([128, 2, 512], FP8, name="junk")
        nc.vector.memset(junk, 0.0)
        junk_lhs = consts.tile([128, 2, 128], FP8, name="junk_lhs")
        nc.vector.memset(junk_lhs, 0.0)
        ident = consts.tile([128, 128], BF16)
        make_identity(nc, ident)

        def junk_mm():
            jps = cpsum_pool.tile([128, 512], F32, name="ps", tag="ps")
            nc.tensor.matmul(jps, junk_lhs, junk, start=True, stop=True,
                             perf_mode=DR)

        for _ in range(6):
            junk_mm()

        # --- DMA issue order is bandwidth-critical: the conv stream can't
        # start until W_c0 + the first x rows are in SBUF (~360 GB/s/core).
        wstage = wstage_pool.tile([128, CO_BLKS, CIN, 9], F32)
        wb = wstage_pool.tile([128, CO_BLKS, CIN, 9], BF16)
        lhst = lhst_pool.tile([128, CI_BLKS, 9, COUT], FP8)
        xstage0 = xstage_pool.tile([128, CI_BLKS, H * W], F32,
                                   name="xstage0", tag="xstage")

        def dma_w(c, b):
            # one quarter of the weights: co block c, ci block b
            nc.sync.dma_start(
                out=wstage[:, c, b * 128:(b + 1) * 128],
                in_=w_ap[c * 128:(c + 1) * 128, b * 128:(b + 1) * 128].rearrange(
                    "co ci kh kw -> co ci (kh kw)"),
            )

        def dma_x(xstage, n, r0, r1, b):
            nc.sync.dma_start(
                out=xstage[:, b, r0 * W:r1 * W],
                in_=x_ap[n, b * 128:(b + 1) * 128, r0:r1]
                    .rearrange("c h w -> c (h w)"),
            )

        # interleave so the bytes gating the first conv chunk arrive first:
        # lhsT(c=0) needs both W_c0 quarters; chunk k=0 needs x rows 0-8 only
        dma_w(0, 0)
        dma_w(0, 1)
        dma_x(xstage0, 0, 0, 9, 0)
        dma_x(xstage0, 0, 0, 9, 1)
        dma_w(1, 0)
        dma_w(1, 1)
        dma_x(xstage0, 0, 9, 18, 0)
        dma_x(xstage0, 0, 9, 18, 1)
        dma_x(xstage0, 0, 18, 28, 0)
        dma_x(xstage0, 0, 18, 28, 1)
        dma_x(xstage0, 0, 28, 42, 0)
        dma_x(xstage0, 0, 28, 42, 1)
        dma_x(xstage0, 0, 42, H, 0)
        dma_x(xstage0, 0, 42, H, 1)
        bias_sb = consts.tile([128, CO_BLKS], F32)
        nc.scalar.dma_start(out=bias_sb, in_=b_ap.rearrange("(b p) -> p b", p=128))

        # --- weight prep (PSUM->SBUF casts on ScalarE); DVE work emitted in
        # DMA-arrival order ---------------------------------------------------
        # lhsT[ci_p, ci_blk, tap, co] in fp8 (cast on the PSUM->SBUF copy)
        def binz(dst, src):
            nc.vector.tensor_scalar(dst, src, 0.0, 0.5, ALU.is_ge, ALU.subtract)

        def wprep(c, b):
            binz(wb[:, c, b * 128:(b + 1) * 128], wstage[:, c, b * 128:(b + 1) * 128])
            for t in range(9):
                tp = tpsum_pool.tile([128, 128], BF16)
                nc.tensor.transpose(tp, wb[:, c, b * 128:(b + 1) * 128, t], ident)
                nc.scalar.copy(out=lhst[:, b, t, c * 128:(c + 1) * 128],
                               in_=tp)

        # --- x buffers: persistent padded buffers, pad zeros written once
        NXPAD = 3
        xpads = [xpad_pool.tile([128, CI_BLKS, FA2], FP8,
                                name=f"xpad{i}", tag=f"xpad{i}")
                 for i in range(NXPAD)]
        for xp in xpads:
            for b in range(CI_BLKS):
                # top pad row q=0 + left pad of row q=1
                nc.vector.memset(xp[:, b, 0:58], 0.0)
                # bottom pad row q=57 + tail guard/align
                nc.vector.memset(xp[:, b, 57 * 57:FA2], 0.0)
                # left-pad col (elem 57q) of rows q=1..56; doubles as the
                # previous row's right pad in the shared-pad layout
                nc.vector.memset(
                    xp[:, b, 57:57 + 56 * WP2].rearrange(
                        "p (h w) -> p h w", w=WP2)[:, :, 0:1],
                    0.0,
                )

        # --- per-image pipeline -------------------------------------------
        def binz_x(xstage, xpad, r0, r1, b):
            # data row h -> elems 58+57h .. 58+57h+55
            dst = xpad[:, b, 58:58 + H * WP2].rearrange(
                "p (h w) -> p h w", w=WP2)[:, r0:r1, 0:W]
            src = xstage[:, b].rearrange("p (h w) -> p h w", w=W)[:, r0:r1]
            binz(dst, src)

        def conv_rows(n, xpad, c, r0, nr):
            # conv for output rows [r0, r0+nr); N = nr*57 - 1 moving positions
            nv = nr * WP2 - 1
            ps = cpsum_pool.tile([128, 456], F32, name="ps", tag="ps")
            for t in range(9):
                kh, kw = divmod(t, 3)
                base = (r0 + kh) * WP2 + kw
                nc.tensor.matmul(
                    ps[:, 0:nv],
                    lhst[:, 0:2, t, c * 128:(c + 1) * 128],
                    xpad[:, 0:2, base:base + nv],
                    start=(t == 0),
                    stop=(t == 8),
                    perf_mode=DR,
                )
            osb = out_pool.tile([128, R * W], F32, name="osb")
            nc.scalar.activation(
                out=osb[:, 0:nr * W].rearrange("p (r w) -> p r w", w=W),
                in_=ps.rearrange("p (r w) -> p r w", w=WP2)[:, 0:nr, 0:56],
                func=AF.Identity,
                bias=bias_sb[:, c:c + 1],
                scale=scale,
            )
            nc.sync.dma_start(
                out=y_ap[n, c * 128:(c + 1) * 128]
                    .rearrange("co h w -> co (h w)")[:, r0 * W:(r0 + nr) * W],
                in_=osb[:, 0:nr * W],
            )

        def conv_chunk(n, xpad, c, k):
            # the very last chunk of the run gates the output-DMA drain in
            # the kernel tail; halve it so the final evict+DMA is shorter
            if n == n_imgs - 1 and c == CO_BLKS - 1 and k == NCHUNK - 1:
                conv_rows(n, xpad, c, R * k, 4)
                conv_rows(n, xpad, c, R * k + 4, 4)
            else:
                conv_rows(n, xpad, c, R * k, R)

        def load_image(n):
            # loads + binarizes image n into its xpad buffer
            xstage = xstage_pool.tile([128, CI_BLKS, H * W], F32,
                                      name=f"xstage{n}", tag="xstage")
            xpad = xpads[n % NXPAD]
            for r0, r1 in ((0, 28), (28, H)):
                for b in range(CI_BLKS):
                    dma_x(xstage, n, r0, r1, b)
                    binz_x(xstage, xpad, r0, r1, b)

        for n in range(n_imgs):
            xpad = xpads[n % NXPAD]
            if n == 0:
                # emission order = engine program order (DMA-arrival order);
                # junk matmuls between wprep groups run in the PE-queue slots
                # where transposes would stall on the DVE binarize, keeping
                # early PE utilization high for the power governor
                wprep(0, 0)
                junk_mm()
                junk_mm()
                wprep(0, 1)
                junk_mm()
                junk_mm()
                binz_x(xstage0, xpad, 0, 9, 0)
                binz_x(xstage0, xpad, 0, 9, 1)
                wprep(1, 0)
                junk_mm()
                junk_mm()
                wprep(1, 1)
                junk_mm()
                junk_mm()
                binz_x(xstage0, xpad, 9, 18, 0)
                binz_x(xstage0, xpad, 9, 18, 1)
                binz_x(xstage0, xpad, 18, 28, 0)
                binz_x(xstage0, xpad, 18, 28, 1)
                binz_x(xstage0, xpad, 28, 42, 0)
                binz_x(xstage0, xpad, 28, 42, 1)
                binz_x(xstage0, xpad, 42, H, 0)
                binz_x(xstage0, xpad, 42, H, 1)
                # no HAM warm-up: sustained fp8-DR conv never triggers the
                # power governor's payback throttle (bf16 junk did), and
                # starting conv early at the cold half-clock beats burning
                # ~8us of junk matmuls first.
            # prefetch image n+1 before image n's conv chunks so its input
            # DMAs take queue priority over image n's output-DMA burst
            if n + 1 < n_imgs:
                load_image(n + 1)
            if n == 0:
                # k-major: image 0's rows are still streaming in from HBM;
                # visiting each row chunk twice (both co blocks) halves the
                # row-consumption rate so the PE never starves behind the DMA
                for k in range(NCHUNK):
                    for c in range(CO_BLKS):
                        conv_chunk(n, xpad, c, k)
            else:
                for c in range(CO_BLKS):
                    for k in range(NCHUNK):
                        conv_chunk(n, xpad, c, k)


ALU = mybir.AluOpType

_NC_CACHE = {}


def _get_nc(n_imgs):
    if n_imgs not in _NC_CACHE:
        nc = bacc.Bacc("TRN2", target_bir_lowering=False, debug=False)
        x_ap = nc.dram_tensor("x", [n_imgs, CIN, H, W], F32,
                              kind="ExternalInput").ap()
        w_ap = nc.dram_tensor("weight", [COUT, CIN, 3, 3], F32,
                              kind="ExternalInput").ap()
        b_ap = nc.dram_tensor("bias", [COUT], F32, kind="ExternalInput").ap()
        y_ap = nc.dram_tensor("y", [n_imgs, COUT, H, W], F32,
                              kind="ExternalOutput").ap()
        with tile.TileContext(nc) as tc:
            _build_conv(tc, y_ap, x_ap, w_ap, b_ap, n_imgs)
        nc.compile()
        _NC_CACHE[n_imgs] = nc
    return _NC_CACHE[n_imgs]


def kernel(x: np.ndarray, weight: np.ndarray, bias: np.ndarray) -> np.ndarray:
    assert x.shape[1:] == (CIN, H, W), x.shape
    assert x.shape[0] % N_CORES == 0, x.shape
    n_imgs = x.shape[0] // N_CORES
    x = np.ascontiguousarray(x, dtype=np.float32)
    weight = np.ascontiguousarray(weight, dtype=np.float32)
    bias = np.ascontiguousarray(bias, dtype=np.float32)

    nc = _get_nc(n_imgs)
    shards = [x[i * n_imgs:(i + 1) * n_imgs] for i in range(N_CORES)]
    in_maps = [{"x": s, "weight": weight, "bias": bias} for s in shards]
    res = run_bass_kernel_spmd(nc, in_maps, core_ids=list(range(N_CORES)))
    return np.concatenate([r["y"] for r in res.results], axis=0)



# revision 35
# speedup vs baseline: 1.2103x; 1.2016x over previous
"""Binarized 3x3 conv (stride 1, pad 1) + bias on 8 Trainium2 NeuronCores.

Full problem: x[32,256,56,56] f32, weight[256,256,3,3] f32, bias[256] f32
-> y[32,256,56,56] f32 with y = conv2d(sign(x), sign(weight), pad=1) + bias
(sign(t) = +1 for t >= 0 else -1).

Sharding: data-parallel over batch. Each of the 8 cores gets 4 images and a
replicated copy of weight/bias, computes its shard fully on-device, and the
host concatenates the 8 output shards.

Per-core kernel:
  - binarize x and w to +/-0.5 with one fused DVE op each ((v>=0) - 0.5);
    the final PSUM->SBUF copy applies scale=4 to undo the 0.25 product
    scale, so results are exactly the +/-1 conv (all integers, exact in f32).
  - x lives zero-padded in SBUF as [128(ci_p), 2(ci_blk), 3312] fp8 per
    image in a shared-pad 57-wide row layout: row q (q=0..57, image row q-1)
    at elems 57q..57q+56; col 0 is the left pad and doubles as the previous
    row's right pad. Pad zeros are written once per buffer; data rows are
    rewritten per image.
  - weight is binarized to bf16, transposed on the PE (36 x 128x128
    transposes via identity), and stored as fp8 lhsT
    [128(ci_p), 2(ci_blk), 9(tap), 256(co)].
  - conv: for each (co_blk, image, 8-row output chunk): accumulate 9
    DoubleRow fp8 matmuls (one per tap, K=256 packed as [128,2]),
    M=128/N=455, into one PSUM bank; issue rate is N/2.4GHz + ~2.5ns with
    LDWEIGHTS fully hidden. One garbage column per row is skipped on the
    way out.
  - PSUM -> SBUF via ScalarE: Identity(psum*4 + bias[co]), then DMA to y.
  - the chip power governor only grants sustained full PE clock ~29us in;
    fp8-DR junk matmuls on zeros from t~0 (never bf16 -- that triggers a
    half-clock payback window) plus a k-major chunk order for image 0 keep
    the PE busy through the ramp so most of it is spent on real work.
"""

import numpy as np

import concourse.bacc as bacc
import concourse.mybir as mybir
import concourse.tile as tile
from concourse.bass_utils import run_bass_kernel_spmd
from concourse.masks import make_identity

F32 = mybir.dt.float32
BF16 = mybir.dt.bfloat16
FP8 = mybir.dt.float8e4
AF = mybir.ActivationFunctionType
ALU = mybir.AluOpType
DR = mybir.MatmulPerfMode.DoubleRow

N_CORES = 8
H = W = 56
WP = 58            # padded row width
CIN = 256
COUT = 256
CI_BLKS = 2        # 256 ci = 2 x 128 partitions
CO_BLKS = 2
R = 8              # output rows per chunk
NCHUNK = H // R    # 7
NV = R * WP        # 464 matmul moving free size
IMG_FA = 3376      # aligned per-ci_blk padded image elems (58*58+2 -> 3376)
WP2 = 57           # shared-pad row width: [pad][56 data], right pad = next row's col 0
NV2 = 455          # 7*57+55+1 moving positions per 8-row chunk
FA2 = 3312         # 57*58 rows + 1 guard, 16-aligned


def _build_conv(tc, y_ap, x_ap, w_ap, b_ap, n_imgs):
    nc = tc.nc
    scale = 4.0  # undo (+/-0.5)*(+/-0.5) = +/-0.25 product scale

    with (
        tc.tile_pool(name="consts", bufs=1) as consts,
        tc.tile_pool(name="wstage", bufs=1) as wstage_pool,
        tc.tile_pool(name="lhst", bufs=1) as lhst_pool,
        tc.tile_pool(name="xstage", bufs=2) as xstage_pool,
        tc.tile_pool(name="xpad", bufs=1) as xpad_pool,
        tc.tile_pool(name="outsb", bufs=3) as out_pool,
        tc.tile_pool(name="tpsum", bufs=2, space="PSUM") as tpsum_pool,
        tc.tile_pool(name="cpsum", bufs=4, space="PSUM") as cpsum_pool,
    ):
        # --- constants -----------------------------------------------------
        # fp8-DR junk matmuls on zeros from t~0: the HAM power governor grants
        # full PE clock only after ~10us of sustained utilization, so start
        # the clock before any DMA lands (bf16 junk would trigger a payback
        # half-clock window mid-conv; the fp8-DR stream does not).
        junk = consts.tile([128, 2, 512], FP8, name="junk")
        nc.vector.memset(junk, 0.0)
        junk_lhs = consts.tile([128, 2, 128], FP8, name="junk_lhs")
        nc.vector.memset(junk_lhs, 0.0)
        ident = consts.tile([128, 128], BF16)
        make_identity(nc, ident)

        def junk_mm():
            jps = cpsum_pool.tile([128, 512], F32, name="ps", tag="ps")
            nc.tensor.matmul(jps, junk_lhs, junk, start=True, stop=True,
                             perf_mode=DR)

        for _ in range(6):
            junk_mm()

        # --- DMA issue order is bandwidth-critical: the conv stream can't
        # start until W_c0 + the first x rows are in SBUF (~360 GB/s/core).
        wstage = wstage_pool.tile([128, CO_BLKS, CIN, 9], F32)
        wb = wstage_pool.tile([128, CO_BLKS, CIN, 9], BF16)
        lhst = lhst_pool.tile([128, CI_BLKS, 9, COUT], FP8)
        xstage0 = xstage_pool.tile([128, CI_BLKS, H * W], F32,
                                   name="xstage0", tag="xstage")

        def dma_w(c, b):
            # one quarter of the weights: co block c, ci block b
            nc.sync.dma_start(
                out=wstage[:, c, b * 128:(b + 1) * 128],
                in_=w_ap[c * 128:(c + 1) * 128, b * 128:(b + 1) * 128].rearrange(
                    "co ci kh kw -> co ci (kh kw)"),
            )

        def dma_x(xstage, n, r0, r1, b):
            nc.sync.dma_start(
                out=xstage[:, b, r0 * W:r1 * W],
                in_=x_ap[n, b * 128:(b + 1) * 128, r0:r1]
                    .rearrange("c h w -> c (h w)"),
            )

        # interleave so the bytes gating the first conv chunk arrive first:
        # lhsT(c=0) needs both W_c0 quarters; chunk k=0 needs x rows 0-8 only
        dma_w(0, 0)
        dma_w(0, 1)
        dma_x(xstage0, 0, 0, 9, 0)
        dma_x(xstage0, 0, 0, 9, 1)
        dma_w(1, 0)
        dma_w(1, 1)
        dma_x(xstage0, 0, 9, 18, 0)
        dma_x(xstage0, 0, 9, 18, 1)
        dma_x(xstage0, 0, 18, 28, 0)
        dma_x(xstage0, 0, 18, 28, 1)
        dma_x(xstage0, 0, 28, 42, 0)
        dma_x(xstage0, 0, 28, 42, 1)
        dma_x(xstage0, 0, 42, H, 0)
        dma_x(xstage0, 0, 42, H, 1)
        bias_sb = consts.tile([128, CO_BLKS], F32)
        nc.scalar.dma_start(out=bias_sb, in_=b_ap.rearrange("(b p) -> p b", p=128))

        # --- weight prep (PSUM->SBUF casts on ScalarE); DVE work emitted in
        # DMA-arrival order ---------------------------------------------------
        # lhsT[ci_p, ci_blk, tap, co] in fp8 (cast on the PSUM->SBUF copy)
        def binz(dst, src):
            nc.vector.tensor_scalar(dst, src, 0.0, 0.5, ALU.is_ge, ALU.subtract)

        def wprep(c, b):
            binz(wb[:, c, b * 128:(b + 1) * 128], wstage[:, c, b * 128:(b + 1) * 128])
            # batch 4 transposed taps per PSUM->SBUF copy: the serial ACT
            # copies are the critical path to the first conv matmul
            for t0 in (0, 4, 8):
                jn = min(4, 9 - t0)
                tp = tpsum_pool.tile([128, 4, 128], BF16)
                for j in range(jn):
                    nc.tensor.transpose(
                        tp[:, j], wb[:, c, b * 128:(b + 1) * 128, t0 + j],
                        ident)
                nc.scalar.copy(
                    out=lhst[:, b, t0:t0 + jn, c * 128:(c + 1) * 128],
                    in_=tp[:, 0:jn])

        # --- x buffers: persistent padded buffers, pad zeros written once
        NXPAD = 3
        xpads = [xpad_pool.tile([128, CI_BLKS, FA2], FP8,
                                name=f"xpad{i}", tag=f"xpad{i}")
                 for i in range(NXPAD)]

        def memset_xpad(xp):
            # pad zeros, written once per buffer (deferred off the startup
            # critical path: DVE must run the first binarizes ASAP)
            for b in range(CI_BLKS):
                # top pad row q=0 + left pad of row q=1
                nc.vector.memset(xp[:, b, 0:58], 0.0)
                # bottom pad row q=57 + tail guard/align
                nc.vector.memset(xp[:, b, 57 * 57:FA2], 0.0)
                # left-pad col (elem 57q) of rows q=1..56; doubles as the
                # previous row's right pad in the shared-pad layout
                nc.vector.memset(
                    xp[:, b, 57:57 + 56 * WP2].rearrange(
                        "p (h w) -> p h w", w=WP2)[:, :, 0:1],
                    0.0,
                )

        # --- per-image pipeline -------------------------------------------
        def binz_x(xstage, xpad, r0, r1, b):
            # data row h -> elems 58+57h .. 58+57h+55
            dst = xpad[:, b, 58:58 + H * WP2].rearrange(
                "p (h w) -> p h w", w=WP2)[:, r0:r1, 0:W]
            src = xstage[:, b].rearrange("p (h w) -> p h w", w=W)[:, r0:r1]
            binz(dst, src)

        def conv_rows(n, xpad, c, r0, nr):
            # conv for output rows [r0, r0+nr); N = nr*57 - 1 moving positions
            nv = nr * WP2 - 1
            ps = cpsum_pool.tile([128, 456], F32, name="ps", tag="ps")
            for t in range(9):
                kh, kw = divmod(t, 3)
                base = (r0 + kh) * WP2 + kw
                nc.tensor.matmul(
                    ps[:, 0:nv],
                    lhst[:, 0:2, t, c * 128:(c + 1) * 128],
                    xpad[:, 0:2, base:base + nv],
                    start=(t == 0),
                    stop=(t == 8),
                    perf_mode=DR,
                )
            osb = out_pool.tile([128, R * W], F32, name="osb")
            nc.scalar.activation(
                out=osb[:, 0:nr * W].rearrange("p (r w) -> p r w", w=W),
                in_=ps.rearrange("p (r w) -> p r w", w=WP2)[:, 0:nr, 0:56],
                func=AF.Identity,
                bias=bias_sb[:, c:c + 1],
                scale=scale,
            )
            nc.sync.dma_start(
                out=y_ap[n, c * 128:(c + 1) * 128]
                    .rearrange("co h w -> co (h w)")[:, r0 * W:(r0 + nr) * W],
                in_=osb[:, 0:nr * W],
            )

        def conv_chunk(n, xpad, c, k):
            # the very last chunk of the run gates the output-DMA drain in
            # the kernel tail; halve it so the final evict+DMA is shorter
            if n == n_imgs - 1 and c == CO_BLKS - 1 and k == NCHUNK - 1:
                conv_rows(n, xpad, c, R * k, 4)
                conv_rows(n, xpad, c, R * k + 4, 4)
            else:
                conv_rows(n, xpad, c, R * k, R)

        def load_image(n):
            # loads + binarizes image n into its xpad buffer
            xstage = xstage_pool.tile([128, CI_BLKS, H * W], F32,
                                      name=f"xstage{n}", tag="xstage")
            xpad = xpads[n % NXPAD]
            for r0, r1 in ((0, 28), (28, H)):
                for b in range(CI_BLKS):
                    dma_x(xstage, n, r0, r1, b)
                    binz_x(xstage, xpad, r0, r1, b)

        for n in range(n_imgs):
            xpad = xpads[n % NXPAD]
            if n == 0:
                # emission order = engine program order (DMA-arrival order);
                # junk matmuls between wprep groups run in the PE-queue slots
                # where transposes would stall on the DVE binarize, keeping
                # early PE utilization high for the power governor
                wprep(0, 0)
                junk_mm()
                junk_mm()
                wprep(0, 1)
                junk_mm()
                junk_mm()
                binz_x(xstage0, xpad, 0, 9, 0)
                binz_x(xstage0, xpad, 0, 9, 1)
                memset_xpad(xpads[0])
                wprep(1, 0)
                junk_mm()
                junk_mm()
                wprep(1, 1)
                junk_mm()
                junk_mm()
                binz_x(xstage0, xpad, 9, 18, 0)
                binz_x(xstage0, xpad, 9, 18, 1)
                binz_x(xstage0, xpad, 18, 28, 0)
                binz_x(xstage0, xpad, 18, 28, 1)
                binz_x(xstage0, xpad, 28, 42, 0)
                binz_x(xstage0, xpad, 28, 42, 1)
                binz_x(xstage0, xpad, 42, H, 0)
                binz_x(xstage0, xpad, 42, H, 1)
                memset_xpad(xpads[1])
                memset_xpad(xpads[2])
                # no HAM warm-up: sustained fp8-DR conv never triggers the
                # power governor's payback throttle (bf16 junk did), and
                # starting conv early at the cold half-clock beats burning
                # ~8us of junk matmuls first.
            # prefetch image n+1 before image n's conv chunks so its input
            # DMAs take queue priority over image n's output-DMA burst
            if n + 1 < n_imgs:
                load_image(n + 1)
            if n == 0:
                # k-major: image 0's rows are still streaming in from HBM;
                # visiting each row chunk twice (both co blocks) halves the
                # row-consumption rate so the PE never starves behind the DMA
                for k in range(NCHUNK):
                    for c in range(CO_BLKS):
                        conv_chunk(n, xpad, c, k)
            else:
                for c in range(CO_BLKS):
                    for k in range(NCHUNK):
                        conv_chunk(n, xpad, c, k)


ALU = mybir.AluOpType

_NC_CACHE = {}


def _get_nc(n_imgs):
    if n_imgs not in _NC_CACHE:
        nc = bacc.Bacc("TRN2", target_bir_lowering=False, debug=False)
        x_ap = nc.dram_tensor("x", [n_imgs, CIN, H, W], F32,
                              kind="ExternalInput").ap()
        w_ap = nc.dram_tensor("weight", [COUT, CIN, 3, 3], F32,
                              kind="ExternalInput").ap()
        b_ap = nc.dram_tensor("bias", [COUT], F32, kind="ExternalInput").ap()
        y_ap = nc.dram_tensor("y", [n_imgs, COUT, H, W], F32,
                              kind="ExternalOutput").ap()
        with tile.TileContext(nc) as tc:
            _build_conv(tc, y_ap, x_ap, w_ap, b_ap, n_imgs)
        nc.compile()
        _NC_CACHE[n_imgs] = nc
    return _NC_CACHE[n_imgs]


def kernel(x: np.ndarray, weight: np.ndarray, bias: np.ndarray) -> np.ndarray:
    assert x.shape[1:] == (CIN, H, W), x.shape
    assert x.shape[0] % N_CORES == 0, x.shape
    n_imgs = x.shape[0] // N_CORES
    x = np.ascontiguousarray(x, dtype=np.float32)
    weight = np.ascontiguousarray(weight, dtype=np.float32)
    bias = np.ascontiguousarray(bias, dtype=np.float32)

    nc = _get_nc(n_imgs)
    shards = [x[i * n_imgs:(i + 1) * n_imgs] for i in range(N_CORES)]
    in_maps = [{"x": s, "weight": weight, "bias": bias} for s in shards]
    res = run_bass_kernel_spmd(nc, in_maps, core_ids=list(range(N_CORES)))
    return np.concatenate([r["y"] for r in res.results], axis=0)



# revision 36
# speedup vs baseline: 1.2123x; 1.0017x over previous
"""Binarized 3x3 conv (stride 1, pad 1) + bias on 8 Trainium2 NeuronCores.

Full problem: x[32,256,56,56] f32, weight[256,256,3,3] f32, bias[256] f32
-> y[32,256,56,56] f32 with y = conv2d(sign(x), sign(weight), pad=1) + bias
(sign(t) = +1 for t >= 0 else -1).

Sharding: data-parallel over batch. Each of the 8 cores gets 4 images and a
replicated copy of weight/bias, computes its shard fully on-device, and the
host concatenates the 8 output shards.

Per-core kernel:
  - binarize x and w to +/-0.5 with one fused DVE op each ((v>=0) - 0.5);
    the final PSUM->SBUF copy applies scale=4 to undo the 0.25 product
    scale, so results are exactly the +/-1 conv (all integers, exact in f32).
  - x lives zero-padded in SBUF as [128(ci_p), 2(ci_blk), 3312] fp8 per
    image in a shared-pad 57-wide row layout: row q (q=0..57, image row q-1)
    at elems 57q..57q+56; col 0 is the left pad and doubles as the previous
    row's right pad. Pad zeros are written once per buffer; data rows are
    rewritten per image.
  - weight is binarized to bf16, transposed on the PE (36 x 128x128
    transposes via identity), and stored as fp8 lhsT
    [128(ci_p), 2(ci_blk), 9(tap), 256(co)].
  - conv: for each (co_blk, image, 8-row output chunk): accumulate 9
    DoubleRow fp8 matmuls (one per tap, K=256 packed as [128,2]),
    M=128/N=455, into one PSUM bank; issue rate is N/2.4GHz + ~2.5ns with
    LDWEIGHTS fully hidden. One garbage column per row is skipped on the
    way out.
  - PSUM -> SBUF via ScalarE: Identity(psum*4 + bias[co]), then DMA to y.
  - the chip power governor only grants sustained full PE clock ~29us in;
    fp8-DR junk matmuls on zeros from t~0 (never bf16 -- that triggers a
    half-clock payback window) plus a k-major chunk order for image 0 keep
    the PE busy through the ramp so most of it is spent on real work.
"""

import numpy as np

import concourse.bacc as bacc
import concourse.mybir as mybir
import concourse.tile as tile
from concourse.bass_utils import run_bass_kernel_spmd
from concourse.masks import make_identity

F32 = mybir.dt.float32
BF16 = mybir.dt.bfloat16
FP8 = mybir.dt.float8e4
AF = mybir.ActivationFunctionType
ALU = mybir.AluOpType
DR = mybir.MatmulPerfMode.DoubleRow

N_CORES = 8
H = W = 56
WP = 58            # padded row width
CIN = 256
COUT = 256
CI_BLKS = 2        # 256 ci = 2 x 128 partitions
CO_BLKS = 2
R = 8              # output rows per chunk
NCHUNK = H // R    # 7
NV = R * WP        # 464 matmul moving free size
IMG_FA = 3376      # aligned per-ci_blk padded image elems (58*58+2 -> 3376)
WP2 = 57           # shared-pad row width: [pad][56 data], right pad = next row's col 0
NV2 = 455          # 7*57+55+1 moving positions per 8-row chunk
FA2 = 3312         # 57*58 rows + 1 guard, 16-aligned


def _build_conv(tc, y_ap, x_ap, w_ap, b_ap, n_imgs):
    nc = tc.nc
    scale = 4.0  # undo (+/-0.5)*(+/-0.5) = +/-0.25 product scale

    with (
        tc.tile_pool(name="consts", bufs=1) as consts,
        tc.tile_pool(name="wstage", bufs=1) as wstage_pool,
        tc.tile_pool(name="lhst", bufs=1) as lhst_pool,
        tc.tile_pool(name="xstage", bufs=2) as xstage_pool,
        tc.tile_pool(name="xpad", bufs=1) as xpad_pool,
        tc.tile_pool(name="outsb", bufs=3) as out_pool,
        tc.tile_pool(name="tpsum", bufs=2, space="PSUM") as tpsum_pool,
        tc.tile_pool(name="cpsum", bufs=4, space="PSUM") as cpsum_pool,
    ):
        # --- constants -----------------------------------------------------
        # fp8-DR junk matmuls on zeros from t~0: the HAM power governor grants
        # full PE clock only after ~10us of sustained utilization, so start
        # the clock before any DMA lands (bf16 junk would trigger a payback
        # half-clock window mid-conv; the fp8-DR stream does not).
        junk = consts.tile([128, 2, 512], FP8, name="junk")
        nc.gpsimd.memset(junk, 0.0)
        junk_lhs = consts.tile([128, 2, 128], FP8, name="junk_lhs")
        nc.gpsimd.memset(junk_lhs, 0.0)
        ident = consts.tile([128, 128], BF16)
        make_identity(nc, ident)

        def junk_mm():
            jps = cpsum_pool.tile([128, 512], F32, name="ps", tag="ps")
            nc.tensor.matmul(jps, junk_lhs, junk, start=True, stop=True,
                             perf_mode=DR)

        for _ in range(6):
            junk_mm()

        # --- DMA issue order is bandwidth-critical: the conv stream can't
        # start until W_c0 + the first x rows are in SBUF (~360 GB/s/core).
        wstage = wstage_pool.tile([128, CO_BLKS, CIN, 9], F32)
        wb = wstage_pool.tile([128, CO_BLKS, CIN, 9], BF16)
        lhst = lhst_pool.tile([128, CI_BLKS, 9, COUT], FP8)
        xstage0 = xstage_pool.tile([128, CI_BLKS, H * W], F32,
                                   name="xstage0", tag="xstage")

        def dma_w(c, b):
            # one quarter of the weights: co block c, ci block b
            nc.sync.dma_start(
                out=wstage[:, c, b * 128:(b + 1) * 128],
                in_=w_ap[c * 128:(c + 1) * 128, b * 128:(b + 1) * 128].rearrange(
                    "co ci kh kw -> co ci (kh kw)"),
            )

        def dma_x(xstage, n, r0, r1, b):
            nc.sync.dma_start(
                out=xstage[:, b, r0 * W:r1 * W],
                in_=x_ap[n, b * 128:(b + 1) * 128, r0:r1]
                    .rearrange("c h w -> c (h w)"),
            )

        # interleave so the bytes gating the first conv chunk arrive first:
        # lhsT(c=0) needs both W_c0 quarters; chunk k=0 needs x rows 0-8 only
        dma_w(0, 0)
        dma_w(0, 1)
        dma_x(xstage0, 0, 0, 9, 0)
        dma_x(xstage0, 0, 0, 9, 1)
        dma_w(1, 0)
        dma_w(1, 1)
        dma_x(xstage0, 0, 9, 18, 0)
        dma_x(xstage0, 0, 9, 18, 1)
        dma_x(xstage0, 0, 18, 28, 0)
        dma_x(xstage0, 0, 18, 28, 1)
        dma_x(xstage0, 0, 28, 42, 0)
        dma_x(xstage0, 0, 28, 42, 1)
        dma_x(xstage0, 0, 42, H, 0)
        dma_x(xstage0, 0, 42, H, 1)
        bias_sb = consts.tile([128, CO_BLKS], F32)
        nc.scalar.dma_start(out=bias_sb, in_=b_ap.rearrange("(b p) -> p b", p=128))

        # --- weight prep (PSUM->SBUF casts on ScalarE); DVE work emitted in
        # DMA-arrival order ---------------------------------------------------
        # lhsT[ci_p, ci_blk, tap, co] in fp8 (cast on the PSUM->SBUF copy)
        def binz(dst, src):
            nc.vector.tensor_scalar(dst, src, 0.0, 0.5, ALU.is_ge, ALU.subtract)

        def wprep(c, b):
            binz(wb[:, c, b * 128:(b + 1) * 128], wstage[:, c, b * 128:(b + 1) * 128])
            # batch 4 transposed taps per PSUM->SBUF copy: the serial ACT
            # copies are the critical path to the first conv matmul
            for t0 in (0, 4, 8):
                jn = min(4, 9 - t0)
                tp = tpsum_pool.tile([128, 4, 128], BF16)
                for j in range(jn):
                    nc.tensor.transpose(
                        tp[:, j], wb[:, c, b * 128:(b + 1) * 128, t0 + j],
                        ident)
                nc.scalar.copy(
                    out=lhst[:, b, t0:t0 + jn, c * 128:(c + 1) * 128],
                    in_=tp[:, 0:jn])

        # --- x buffers: persistent padded buffers, pad zeros written once
        NXPAD = 3
        xpads = [xpad_pool.tile([128, CI_BLKS, FA2], FP8,
                                name=f"xpad{i}", tag=f"xpad{i}")
                 for i in range(NXPAD)]

        def memset_xpad(xp):
            # pad zeros, written once per buffer (deferred off the startup
            # critical path: DVE must run the first binarizes ASAP)
            for b in range(CI_BLKS):
                # top pad row q=0 + left pad of row q=1
                nc.vector.memset(xp[:, b, 0:58], 0.0)
                # bottom pad row q=57 + tail guard/align
                nc.vector.memset(xp[:, b, 57 * 57:FA2], 0.0)
                # left-pad col (elem 57q) of rows q=1..56; doubles as the
                # previous row's right pad in the shared-pad layout
                nc.vector.memset(
                    xp[:, b, 57:57 + 56 * WP2].rearrange(
                        "p (h w) -> p h w", w=WP2)[:, :, 0:1],
                    0.0,
                )

        # --- per-image pipeline -------------------------------------------
        def binz_x(xstage, xpad, r0, r1, b):
            # data row h -> elems 58+57h .. 58+57h+55
            dst = xpad[:, b, 58:58 + H * WP2].rearrange(
                "p (h w) -> p h w", w=WP2)[:, r0:r1, 0:W]
            src = xstage[:, b].rearrange("p (h w) -> p h w", w=W)[:, r0:r1]
            binz(dst, src)

        def conv_rows(n, xpad, c, r0, nr):
            # conv for output rows [r0, r0+nr); N = nr*57 - 1 moving positions
            nv = nr * WP2 - 1
            ps = cpsum_pool.tile([128, 456], F32, name="ps", tag="ps")
            for t in range(9):
                kh, kw = divmod(t, 3)
                base = (r0 + kh) * WP2 + kw
                nc.tensor.matmul(
                    ps[:, 0:nv],
                    lhst[:, 0:2, t, c * 128:(c + 1) * 128],
                    xpad[:, 0:2, base:base + nv],
                    start=(t == 0),
                    stop=(t == 8),
                    perf_mode=DR,
                )
            osb = out_pool.tile([128, R * W], F32, name="osb")
            nc.scalar.activation(
                out=osb[:, 0:nr * W].rearrange("p (r w) -> p r w", w=W),
                in_=ps.rearrange("p (r w) -> p r w", w=WP2)[:, 0:nr, 0:56],
                func=AF.Identity,
                bias=bias_sb[:, c:c + 1],
                scale=scale,
            )
            nc.sync.dma_start(
                out=y_ap[n, c * 128:(c + 1) * 128]
                    .rearrange("co h w -> co (h w)")[:, r0 * W:(r0 + nr) * W],
                in_=osb[:, 0:nr * W],
            )

        def conv_chunk(n, xpad, c, k):
            # the very last chunk of the run gates the output-DMA drain in
            # the kernel tail; halve it so the final evict+DMA is shorter
            if n == n_imgs - 1 and c == CO_BLKS - 1 and k == NCHUNK - 1:
                conv_rows(n, xpad, c, R * k, 4)
                conv_rows(n, xpad, c, R * k + 4, 4)
            else:
                conv_rows(n, xpad, c, R * k, R)

        def load_image(n):
            # loads + binarizes image n into its xpad buffer
            xstage = xstage_pool.tile([128, CI_BLKS, H * W], F32,
                                      name=f"xstage{n}", tag="xstage")
            xpad = xpads[n % NXPAD]
            for r0, r1 in ((0, 28), (28, H)):
                for b in range(CI_BLKS):
                    dma_x(xstage, n, r0, r1, b)
                    binz_x(xstage, xpad, r0, r1, b)

        for n in range(n_imgs):
            xpad = xpads[n % NXPAD]
            if n == 0:
                # emission order = engine program order (DMA-arrival order);
                # junk matmuls between wprep groups run in the PE-queue slots
                # where transposes would stall on the DVE binarize, keeping
                # early PE utilization high for the power governor
                wprep(0, 0)
                junk_mm()
                junk_mm()
                wprep(0, 1)
                junk_mm()
                junk_mm()
                binz_x(xstage0, xpad, 0, 9, 0)
                binz_x(xstage0, xpad, 0, 9, 1)
                memset_xpad(xpads[0])
                wprep(1, 0)
                junk_mm()
                junk_mm()
                wprep(1, 1)
                junk_mm()
                junk_mm()
                binz_x(xstage0, xpad, 9, 18, 0)
                binz_x(xstage0, xpad, 9, 18, 1)
                binz_x(xstage0, xpad, 18, 28, 0)
                binz_x(xstage0, xpad, 18, 28, 1)
                binz_x(xstage0, xpad, 28, 42, 0)
                binz_x(xstage0, xpad, 28, 42, 1)
                binz_x(xstage0, xpad, 42, H, 0)
                binz_x(xstage0, xpad, 42, H, 1)
                memset_xpad(xpads[1])
                memset_xpad(xpads[2])
                # no HAM warm-up: sustained fp8-DR conv never triggers the
                # power governor's payback throttle (bf16 junk did), and
                # starting conv early at the cold half-clock beats burning
                # ~8us of junk matmuls first.
            # prefetch image n+1 before image n's conv chunks so its input
            # DMAs take queue priority over image n's output-DMA burst
            if n + 1 < n_imgs:
                load_image(n + 1)
            if n == 0:
                # k-major: image 0's rows are still streaming in from HBM;
                # visiting each row chunk twice (both co blocks) halves the
                # row-consumption rate so the PE never starves behind the DMA
                for k in range(NCHUNK):
                    for c in range(CO_BLKS):
                        conv_chunk(n, xpad, c, k)
            else:
                for c in range(CO_BLKS):
                    for k in range(NCHUNK):
                        conv_chunk(n, xpad, c, k)


ALU = mybir.AluOpType

_NC_CACHE = {}


def _get_nc(n_imgs):
    if n_imgs not in _NC_CACHE:
        nc = bacc.Bacc("TRN2", target_bir_lowering=False, debug=False)
        x_ap = nc.dram_tensor("x", [n_imgs, CIN, H, W], F32,
                              kind="ExternalInput").ap()
        w_ap = nc.dram_tensor("weight", [COUT, CIN, 3, 3], F32,
                              kind="ExternalInput").ap()
        b_ap = nc.dram_tensor("bias", [COUT], F32, kind="ExternalInput").ap()
        y_ap = nc.dram_tensor("y", [n_imgs, COUT, H, W], F32,
                              kind="ExternalOutput").ap()
        with tile.TileContext(nc) as tc:
            _build_conv(tc, y_ap, x_ap, w_ap, b_ap, n_imgs)
        nc.compile()
        _NC_CACHE[n_imgs] = nc
    return _NC_CACHE[n_imgs]


def kernel(x: np.ndarray, weight: np.ndarray, bias: np.ndarray) -> np.ndarray:
    assert x.shape[1:] == (CIN, H, W), x.shape
    assert x.shape[0] % N_CORES == 0, x.shape
    n_imgs = x.shape[0] // N_CORES
    x = np.ascontiguousarray(x, dtype=np.float32)
    weight = np.ascontiguousarray(weight, dtype=np.float32)
    bias = np.ascontiguousarray(bias, dtype=np.float32)

    nc = _get_nc(n_imgs)
    shards = [x[i * n_imgs:(i + 1) * n_imgs] for i in range(N_CORES)]
    in_maps = [{"x": s, "weight": weight, "bias": bias} for s in shards]
    res = run_bass_kernel_spmd(nc, in_maps, core_ids=list(range(N_CORES)))
    return np.concatenate([r["y"] for r in res.results], axis=0)



# revision 37
# speedup vs baseline: 1.2132x; 1.0008x over previous
"""Binarized 3x3 conv (stride 1, pad 1) + bias on 8 Trainium2 NeuronCores.

Full problem: x[32,256,56,56] f32, weight[256,256,3,3] f32, bias[256] f32
-> y[32,256,56,56] f32 with y = conv2d(sign(x), sign(weight), pad=1) + bias
(sign(t) = +1 for t >= 0 else -1).

Sharding: data-parallel over batch. Each of the 8 cores gets 4 images and a
replicated copy of weight/bias, computes its shard fully on-device, and the
host concatenates the 8 output shards.

Per-core kernel:
  - binarize x and w to +/-0.5 with one fused DVE op each ((v>=0) - 0.5);
    the final PSUM->SBUF copy applies scale=4 to undo the 0.25 product
    scale, so results are exactly the +/-1 conv (all integers, exact in f32).
  - x lives zero-padded in SBUF as [128(ci_p), 2(ci_blk), 3312] fp8 per
    image in a shared-pad 57-wide row layout: row q (q=0..57, image row q-1)
    at elems 57q..57q+56; col 0 is the left pad and doubles as the previous
    row's right pad. Pad zeros are written once per buffer; data rows are
    rewritten per image.
  - weight is binarized to bf16, transposed on the PE (36 x 128x128
    transposes via identity), and stored as fp8 lhsT
    [128(ci_p), 2(ci_blk), 9(tap), 256(co)].
  - conv: for each (co_blk, image, 8-row output chunk): accumulate 9
    DoubleRow fp8 matmuls (one per tap, K=256 packed as [128,2]),
    M=128/N=455, into one PSUM bank; issue rate is N/2.4GHz + ~2.5ns with
    LDWEIGHTS fully hidden. One garbage column per row is skipped on the
    way out.
  - PSUM -> SBUF via ScalarE: Identity(psum*4 + bias[co]), then DMA to y.
  - the chip power governor only grants sustained full PE clock ~29us in;
    fp8-DR junk matmuls on zeros from t~0 (never bf16 -- that triggers a
    half-clock payback window) plus a k-major chunk order for image 0 keep
    the PE busy through the ramp so most of it is spent on real work.
"""

import numpy as np

import concourse.bacc as bacc
import concourse.mybir as mybir
import concourse.tile as tile
from concourse.bass_utils import run_bass_kernel_spmd
from concourse.masks import make_identity

F32 = mybir.dt.float32
BF16 = mybir.dt.bfloat16
FP8 = mybir.dt.float8e4
AF = mybir.ActivationFunctionType
ALU = mybir.AluOpType
DR = mybir.MatmulPerfMode.DoubleRow

N_CORES = 8
H = W = 56
WP = 58            # padded row width
CIN = 256
COUT = 256
CI_BLKS = 2        # 256 ci = 2 x 128 partitions
CO_BLKS = 2
R = 8              # output rows per chunk
NCHUNK = H // R    # 7
NV = R * WP        # 464 matmul moving free size
IMG_FA = 3376      # aligned per-ci_blk padded image elems (58*58+2 -> 3376)
WP2 = 57           # shared-pad row width: [pad][56 data], right pad = next row's col 0
NV2 = 455          # 7*57+55+1 moving positions per 8-row chunk
FA2 = 3312         # 57*58 rows + 1 guard, 16-aligned


def _build_conv(tc, y_ap, x_ap, w_ap, b_ap, n_imgs):
    nc = tc.nc
    scale = 4.0  # undo (+/-0.5)*(+/-0.5) = +/-0.25 product scale

    with (
        tc.tile_pool(name="consts", bufs=1) as consts,
        tc.tile_pool(name="wstage", bufs=1) as wstage_pool,
        tc.tile_pool(name="lhst", bufs=1) as lhst_pool,
        tc.tile_pool(name="xstage", bufs=2) as xstage_pool,
        tc.tile_pool(name="xpad", bufs=1) as xpad_pool,
        tc.tile_pool(name="outsb", bufs=8) as out_pool,
        tc.tile_pool(name="tpsum", bufs=2, space="PSUM") as tpsum_pool,
        tc.tile_pool(name="cpsum", bufs=4, space="PSUM") as cpsum_pool,
    ):
        # --- constants -----------------------------------------------------
        # fp8-DR junk matmuls on zeros from t~0: the HAM power governor grants
        # full PE clock only after ~10us of sustained utilization, so start
        # the clock before any DMA lands (bf16 junk would trigger a payback
        # half-clock window mid-conv; the fp8-DR stream does not).
        junk = consts.tile([128, 2, 512], FP8, name="junk")
        nc.gpsimd.memset(junk, 0.0)
        junk_lhs = consts.tile([128, 2, 128], FP8, name="junk_lhs")
        nc.gpsimd.memset(junk_lhs, 0.0)
        ident = consts.tile([128, 128], BF16)
        make_identity(nc, ident)

        def junk_mm():
            jps = cpsum_pool.tile([128, 512], F32, name="ps", tag="ps")
            nc.tensor.matmul(jps, junk_lhs, junk, start=True, stop=True,
                             perf_mode=DR)

        for _ in range(6):
            junk_mm()

        # --- DMA issue order is bandwidth-critical: the conv stream can't
        # start until W_c0 + the first x rows are in SBUF (~360 GB/s/core).
        wstage = wstage_pool.tile([128, CO_BLKS, CIN, 9], F32)
        wb = wstage_pool.tile([128, CO_BLKS, CIN, 9], BF16)
        lhst = lhst_pool.tile([128, CI_BLKS, 9, COUT], FP8)
        xstage0 = xstage_pool.tile([128, CI_BLKS, H * W], F32,
                                   name="xstage0", tag="xstage")

        def dma_w(c, b):
            # one quarter of the weights: co block c, ci block b
            nc.sync.dma_start(
                out=wstage[:, c, b * 128:(b + 1) * 128],
                in_=w_ap[c * 128:(c + 1) * 128, b * 128:(b + 1) * 128].rearrange(
                    "co ci kh kw -> co ci (kh kw)"),
            )

        def dma_x(xstage, n, r0, r1, b):
            nc.sync.dma_start(
                out=xstage[:, b, r0 * W:r1 * W],
                in_=x_ap[n, b * 128:(b + 1) * 128, r0:r1]
                    .rearrange("c h w -> c (h w)"),
            )

        # interleave so the bytes gating the first conv chunk arrive first:
        # lhsT(c=0) needs both W_c0 quarters; chunk k=0 needs x rows 0-8 only
        dma_w(0, 0)
        dma_w(0, 1)
        dma_x(xstage0, 0, 0, 9, 0)
        dma_x(xstage0, 0, 0, 9, 1)
        dma_w(1, 0)
        dma_w(1, 1)
        dma_x(xstage0, 0, 9, 18, 0)
        dma_x(xstage0, 0, 9, 18, 1)
        dma_x(xstage0, 0, 18, 28, 0)
        dma_x(xstage0, 0, 18, 28, 1)
        dma_x(xstage0, 0, 28, 42, 0)
        dma_x(xstage0, 0, 28, 42, 1)
        dma_x(xstage0, 0, 42, H, 0)
        dma_x(xstage0, 0, 42, H, 1)
        bias_sb = consts.tile([128, CO_BLKS], F32)
        nc.scalar.dma_start(out=bias_sb, in_=b_ap.rearrange("(b p) -> p b", p=128))

        # --- weight prep (PSUM->SBUF casts on ScalarE); DVE work emitted in
        # DMA-arrival order ---------------------------------------------------
        # lhsT[ci_p, ci_blk, tap, co] in fp8 (cast on the PSUM->SBUF copy)
        def binz(dst, src):
            nc.vector.tensor_scalar(dst, src, 0.0, 0.5, ALU.is_ge, ALU.subtract)

        def wprep(c, b):
            binz(wb[:, c, b * 128:(b + 1) * 128], wstage[:, c, b * 128:(b + 1) * 128])
            # batch 4 transposed taps per PSUM->SBUF copy: the serial ACT
            # copies are the critical path to the first conv matmul
            for t0 in (0, 4, 8):
                jn = min(4, 9 - t0)
                tp = tpsum_pool.tile([128, 4, 128], BF16)
                for j in range(jn):
                    nc.tensor.transpose(
                        tp[:, j], wb[:, c, b * 128:(b + 1) * 128, t0 + j],
                        ident)
                nc.scalar.copy(
                    out=lhst[:, b, t0:t0 + jn, c * 128:(c + 1) * 128],
                    in_=tp[:, 0:jn])

        # --- x buffers: persistent padded buffers, pad zeros written once
        NXPAD = 3
        xpads = [xpad_pool.tile([128, CI_BLKS, FA2], FP8,
                                name=f"xpad{i}", tag=f"xpad{i}")
                 for i in range(NXPAD)]

        def memset_xpad(xp):
            # pad zeros, written once per buffer (deferred off the startup
            # critical path: DVE must run the first binarizes ASAP)
            for b in range(CI_BLKS):
                # top pad row q=0 + left pad of row q=1
                nc.vector.memset(xp[:, b, 0:58], 0.0)
                # bottom pad row q=57 + tail guard/align
                nc.vector.memset(xp[:, b, 57 * 57:FA2], 0.0)
                # left-pad col (elem 57q) of rows q=1..56; doubles as the
                # previous row's right pad in the shared-pad layout
                nc.vector.memset(
                    xp[:, b, 57:57 + 56 * WP2].rearrange(
                        "p (h w) -> p h w", w=WP2)[:, :, 0:1],
                    0.0,
                )

        # --- per-image pipeline -------------------------------------------
        def binz_x(xstage, xpad, r0, r1, b):
            # data row h -> elems 58+57h .. 58+57h+55
            dst = xpad[:, b, 58:58 + H * WP2].rearrange(
                "p (h w) -> p h w", w=WP2)[:, r0:r1, 0:W]
            src = xstage[:, b].rearrange("p (h w) -> p h w", w=W)[:, r0:r1]
            binz(dst, src)

        def conv_rows(n, xpad, c, r0, nr):
            # conv for output rows [r0, r0+nr); N = nr*57 - 1 moving positions
            nv = nr * WP2 - 1
            ps = cpsum_pool.tile([128, 456], F32, name="ps", tag="ps")
            for t in range(9):
                kh, kw = divmod(t, 3)
                base = (r0 + kh) * WP2 + kw
                nc.tensor.matmul(
                    ps[:, 0:nv],
                    lhst[:, 0:2, t, c * 128:(c + 1) * 128],
                    xpad[:, 0:2, base:base + nv],
                    start=(t == 0),
                    stop=(t == 8),
                    perf_mode=DR,
                )
            osb = out_pool.tile([128, R * W], F32, name="osb")
            nc.scalar.activation(
                out=osb[:, 0:nr * W].rearrange("p (r w) -> p r w", w=W),
                in_=ps.rearrange("p (r w) -> p r w", w=WP2)[:, 0:nr, 0:56],
                func=AF.Identity,
                bias=bias_sb[:, c:c + 1],
                scale=scale,
            )
            nc.sync.dma_start(
                out=y_ap[n, c * 128:(c + 1) * 128]
                    .rearrange("co h w -> co (h w)")[:, r0 * W:(r0 + nr) * W],
                in_=osb[:, 0:nr * W],
            )

        def conv_chunk(n, xpad, c, k):
            # the very last chunk of the run gates the output-DMA drain in
            # the kernel tail; halve it so the final evict+DMA is shorter
            if n == n_imgs - 1 and c == CO_BLKS - 1 and k == NCHUNK - 1:
                conv_rows(n, xpad, c, R * k, 4)
                conv_rows(n, xpad, c, R * k + 4, 4)
            else:
                conv_rows(n, xpad, c, R * k, R)

        def load_image(n):
            # loads + binarizes image n into its xpad buffer
            xstage = xstage_pool.tile([128, CI_BLKS, H * W], F32,
                                      name=f"xstage{n}", tag="xstage")
            xpad = xpads[n % NXPAD]
            for r0, r1 in ((0, 28), (28, H)):
                for b in range(CI_BLKS):
                    dma_x(xstage, n, r0, r1, b)
                    binz_x(xstage, xpad, r0, r1, b)

        for n in range(n_imgs):
            xpad = xpads[n % NXPAD]
            if n == 0:
                # emission order = engine program order (DMA-arrival order);
                # junk matmuls between wprep groups run in the PE-queue slots
                # where transposes would stall on the DVE binarize, keeping
                # early PE utilization high for the power governor
                wprep(0, 0)
                junk_mm()
                junk_mm()
                wprep(0, 1)
                junk_mm()
                junk_mm()
                binz_x(xstage0, xpad, 0, 9, 0)
                binz_x(xstage0, xpad, 0, 9, 1)
                memset_xpad(xpads[0])
                wprep(1, 0)
                junk_mm()
                junk_mm()
                wprep(1, 1)
                junk_mm()
                junk_mm()
                binz_x(xstage0, xpad, 9, 18, 0)
                binz_x(xstage0, xpad, 9, 18, 1)
                binz_x(xstage0, xpad, 18, 28, 0)
                binz_x(xstage0, xpad, 18, 28, 1)
                binz_x(xstage0, xpad, 28, 42, 0)
                binz_x(xstage0, xpad, 28, 42, 1)
                binz_x(xstage0, xpad, 42, H, 0)
                binz_x(xstage0, xpad, 42, H, 1)
                memset_xpad(xpads[1])
                memset_xpad(xpads[2])
                # no HAM warm-up: sustained fp8-DR conv never triggers the
                # power governor's payback throttle (bf16 junk did), and
                # starting conv early at the cold half-clock beats burning
                # ~8us of junk matmuls first.
            # prefetch image n+1 before image n's conv chunks so its input
            # DMAs take queue priority over image n's output-DMA burst
            if n + 1 < n_imgs:
                load_image(n + 1)
            if n == 0:
                # k-major: image 0's rows are still streaming in from HBM;
                # visiting each row chunk twice (both co blocks) halves the
                # row-consumption rate so the PE never starves behind the DMA
                for k in range(NCHUNK):
                    for c in range(CO_BLKS):
                        conv_chunk(n, xpad, c, k)
            else:
                for c in range(CO_BLKS):
                    for k in range(NCHUNK):
                        conv_chunk(n, xpad, c, k)


ALU = mybir.AluOpType

_NC_CACHE = {}


def _get_nc(n_imgs):
    if n_imgs not in _NC_CACHE:
        nc = bacc.Bacc("TRN2", target_bir_lowering=False, debug=False)
        x_ap = nc.dram_tensor("x", [n_imgs, CIN, H, W], F32,
                              kind="ExternalInput").ap()
        w_ap = nc.dram_tensor("weight", [COUT, CIN, 3, 3], F32,
                              kind="ExternalInput").ap()
        b_ap = nc.dram_tensor("bias", [COUT], F32, kind="ExternalInput").ap()
        y_ap = nc.dram_tensor("y", [n_imgs, COUT, H, W], F32,
                              kind="ExternalOutput").ap()
        with tile.TileContext(nc) as tc:
            _build_conv(tc, y_ap, x_ap, w_ap, b_ap, n_imgs)
        nc.compile()
        _NC_CACHE[n_imgs] = nc
    return _NC_CACHE[n_imgs]


def kernel(x: np.ndarray, weight: np.ndarray, bias: np.ndarray) -> np.ndarray:
    assert x.shape[1:] == (CIN, H, W), x.shape
    assert x.shape[0] % N_CORES == 0, x.shape
    n_imgs = x.shape[0] // N_CORES
    x = np.ascontiguousarray(x, dtype=np.float32)
    weight = np.ascontiguousarray(weight, dtype=np.float32)
    bias = np.ascontiguousarray(bias, dtype=np.float32)

    nc = _get_nc(n_imgs)
    shards = [x[i * n_imgs:(i + 1) * n_imgs] for i in range(N_CORES)]
    in_maps = [{"x": s, "weight": weight, "bias": bias} for s in shards]
    res = run_bass_kernel_spmd(nc, in_maps, core_ids=list(range(N_CORES)))
    return np.concatenate([r["y"] for r in res.results], axis=0)

